# revision 12
# baseline (speedup 1.0000x reference)
"""CLIP causal attention (B=8, T=1024, E=768, H=12) on 8 TRN2 NeuronCores.

Strategy: pure data-parallel over batch — core b handles x[b] end to end,
no collectives. All compute in transposed space (embed on partitions):

  X' = x_b^T                       [768, 1024]  (host pre-transposed, bf16)
  Q' = Wq^T @ X' (+bq)             [768, 1024]  lhsT = Wq as stored
  K' = Wk^T @ X' (+bk)             [768, 1024]
  V  = X'^T @ Wv (+bv)             [1024, 768]  lhsT = X' blocks (j on partitions)
  per head h (KQ orientation, j on partitions, i free):
     S'[j,i] = K'_h[:,jblk]^T @ Q'_h          (K=64)
     P' = exp(S' * 1/8)  (no max-subtraction: |S'/8| <= ~7, exact-safe)
     causal: skip fully-masked blocks, restrict to valid cols, tri-mask diag
     O_aug[d,i] = sum_j Vaug_h[j,d]^T @ P'    (Vaug has a ones column ->
                                               row 64 = softmax denominator)
     O'_h = O_aug[0:64] * broadcast(1/denom)
  out = (O'^T @ Wo) + bo           [1024, 768]  lhsT = O' blocks -> direct
                                                untransposed output

Q/K projections are interleaved with the attention heads that consume them:
projection matmuls (PE-only) fill the TensorE while ScalarE streams the exp()
of earlier heads, keeping PE duty high enough that the HAM clock gate stays
at 2.4GHz. All matmul operands bf16 (fp32 PSUM accumulation); measured
end-to-end rel l2 err vs fp32 reference ~5e-3.
"""

import numpy as np
import ml_dtypes

E = 768
T = 1024
B = 8
H = 12
DH = 64
NT = E // 128          # 6 partition-tiles of the embed dim
NJ = T // 128          # 8 partition-tiles of the token dim
SCALE = DH ** -0.5     # folded into the exp() activation's scale operand
VW = H * 128           # V_aug row width: 12 heads x 128 cols (64 data +
                       # ones col + zero pad so the PV stationary operand
                       # is a full 128x128 block -> fast weight load)

_CACHE = {}


def _build():
    import concourse.bass as bass
    import concourse.tile as tile
    from concourse import bacc, mybir

    f32 = mybir.dt.float32
    bf16 = mybir.dt.bfloat16
    Exp = mybir.ActivationFunctionType.Exp

    nc = bacc.Bacc(
        "TRN2",
        target_bir_lowering=False,
        debug=False,
        enable_asserts=False,
        num_devices=B,
    )

    xt = nc.dram_tensor("xt", [E, T], bf16, kind="ExternalInput").ap()
    wq = nc.dram_tensor("wq", [E, E], bf16, kind="ExternalInput").ap()
    wk = nc.dram_tensor("wk", [E, E], bf16, kind="ExternalInput").ap()
    wv = nc.dram_tensor("wv", [E, E], bf16, kind="ExternalInput").ap()
    wo = nc.dram_tensor("wo", [E, E], bf16, kind="ExternalInput").ap()
    bqt = nc.dram_tensor("bqt", [128, NT], f32, kind="ExternalInput").ap()
    bkt = nc.dram_tensor("bkt", [128, NT], f32, kind="ExternalInput").ap()
    bvr = nc.dram_tensor("bvr", [1, E], bf16, kind="ExternalInput").ap()
    bor = nc.dram_tensor("bor", [1, E], bf16, kind="ExternalInput").ap()
    out = nc.dram_tensor("out", [T, E], f32, kind="ExternalOutput").ap()

    with tile.TileContext(nc) as tc:
        with (
            tc.tile_pool(name="const", bufs=1) as cpool,
            tc.tile_pool(name="psb", bufs=6) as ppool,
            tc.tile_pool(name="rsb", bufs=3) as rpool,
            tc.tile_pool(name="rbsb", bufs=3) as rbpool,
            tc.tile_pool(name="fin", bufs=2) as fpool,
            tc.tile_pool(name="pp", bufs=2, space="PSUM") as pp,
            tc.tile_pool(name="sp", bufs=2, space="PSUM") as sp,
            tc.tile_pool(name="op", bufs=2, space="PSUM") as op,
        ):
            XT = cpool.tile([128, NT * T], bf16)     # (kt, i)
            WQ = cpool.tile([128, NT * E], bf16)     # (kt, n)
            WK = cpool.tile([128, NT * E], bf16)
            WV = cpool.tile([128, NT * E], bf16)
            WO = cpool.tile([128, NT * E], bf16)
            QS = cpool.tile([128, NT * T], bf16)     # Q' (nt, i)
            KS = cpool.tile([128, NT * T], bf16)
            VS = cpool.tile([128, NJ * VW], bf16)    # (jt, h*65+d), col 64 = ones
            OS = cpool.tile([128, NT * T], bf16)     # O' (et, i)
            BQ = cpool.tile([128, NT], f32)
            BK = cpool.tile([128, NT], f32)
            BVR = cpool.tile([1, E], bf16)
            BOR = cpool.tile([1, E], bf16)
            ONE = cpool.tile([1, 128], bf16)

            # ---- input DMAs: tiny constants first (Q/K/V evictions need the
            # biases; don't queue them behind 4.5MB of weights), then
            # per-k-tile splits so compute can start early. WV before WQ/WK:
            # the V projection (which only needs XT + WV) runs first. ----
            nc.sync.dma_start(BQ[:], bqt)
            nc.sync.dma_start(BK[:], bkt)
            nc.sync.dma_start(BVR[:], bvr)
            nc.sync.dma_start(BOR[:], bor)
            xt3 = xt.rearrange("(k p) i -> p k i", p=128)
            w3 = {
                id(WQ): wq.rearrange("(k p) n -> p k n", p=128),
                id(WK): wk.rearrange("(k p) n -> p k n", p=128),
                id(WV): wv.rearrange("(k p) n -> p k n", p=128),
                id(WO): wo.rearrange("(k p) n -> p k n", p=128),
            }
            for kt in range(NT):
                nc.sync.dma_start(XT[:, kt * T : (kt + 1) * T], xt3[:, kt])
                nc.sync.dma_start(WV[:, kt * E : (kt + 1) * E], w3[id(WV)][:, kt])
            for W in (WQ, WK, WO):
                for kt in range(NT):
                    nc.sync.dma_start(W[:, kt * E : (kt + 1) * E], w3[id(W)][:, kt])
            nc.vector.memset(ONE[:], 1.0)
            # V_aug: zero the pad columns once, then set the ones column
            nc.vector.memset(VS[:], 0.0)
            for jt in range(NJ):
                nc.vector.memset(
                    VS[:, jt * VW : (jt + 1) * VW]
                    .rearrange("p (h e) -> p h e", e=128)[:, :, 64:65],
                    1.0,
                )

            # ---- PE warmup: ~5us of dummy matmuls (no DMA dependency) so
            # the HAM activity monitor lifts the 1.2GHz cold gate before the
            # real work arrives ----
            DUM = cpool.tile([1, 512], bf16)
            nc.vector.memset(DUM[:], 1.0)
            w_ps = op.tile([128, 512], f32, tag="oaug")
            for _ in range(12):
                nc.tensor.matmul(
                    w_ps[0:1, :], lhsT=DUM[:, 0:1], rhs=DUM[:], start=True, stop=True
                )

            # ---- V projection: lhsT = X'[kt, jblk] -> V[j, e] + ones-col layout
            for jt in range(NJ):
                for e0, ew, h0, nh in ((0, 512, 0, 8), (512, 256, 8, 4)):
                    ps = pp.tile([128, 512], f32, tag="proj")
                    for kt in range(NT):
                        nc.tensor.matmul(
                            ps[:, :ew],
                            lhsT=XT[:, kt * T + jt * 128 : kt * T + jt * 128 + 128],
                            rhs=WV[:, kt * E + e0 : kt * E + e0 + ew],
                            start=(kt == 0),
                            stop=False,
                        )
                    nc.tensor.matmul(  # += 1 x bv  (bias along free dim)
                        ps[:, :ew],
                        lhsT=ONE[:, 0:128],
                        rhs=BVR[:, e0 : e0 + ew],
                        start=False,
                        stop=True,
                    )
                    dst = (
                        VS[:, jt * VW + h0 * 128 : jt * VW + (h0 + nh) * 128]
                        .rearrange("p (h e) -> p h e", e=128)[:, :, 0:64]
                    )
                    nc.vector.tensor_copy(
                        dst, ps[:, :ew].rearrange("p (h d) -> p h d", d=64)
                    )

            # ---- Q'/K' projection for one 128-row block nt (2 heads) ----
            def qk_proj(nt):
                for W, Bb, DST in ((WQ, BQ, QS), (WK, BK, KS)):
                    for ic in range(2):
                        ps = pp.tile([128, 512], f32, tag="proj")
                        for kt in range(NT):
                            nc.tensor.matmul(
                                ps[:],
                                lhsT=W[:, kt * E + nt * 128 : kt * E + nt * 128 + 128],
                                rhs=XT[:, kt * T + ic * 512 : kt * T + ic * 512 + 512],
                                start=(kt == 0),
                                stop=(kt == NT - 1),
                            )
                        nc.vector.tensor_scalar_add(
                            DST[:, nt * T + ic * 512 : nt * T + ic * 512 + 512],
                            ps[:],
                            Bb[:, nt : nt + 1],
                        )

            def normalize(o_ps, h, ic):
                # softmax denominators live in row 64 (the V_aug ones column).
                # Full-precision reciprocal costs 3.35us on DVE; the ~18-bit
                # approx is plenty, but its BITWISE_NOT seed needs an SBUF
                # operand on hardware, so stage the PSUM row out first.
                nt, po = h // 2, (h % 2) * 64
                dn = rpool.tile([1, 512], f32, tag="denom")
                nc.vector.tensor_copy(dn[:], o_ps[64:65, :])
                r = rpool.tile([1, 512], f32, tag="recip")
                nc.vector.reciprocal_approx_fast(r[:], dn[:])
                rb = rbpool.tile([64, 512], f32, tag="recipb")
                nc.gpsimd.partition_broadcast(rb[:], r[:])
                nc.vector.tensor_mul(
                    OS[po : po + 64, nt * T + ic * 512 : nt * T + ic * 512 + 512],
                    o_ps[0:64, :],
                    rb[:],
                )

            # ---- attention for one head, KQ orientation. Per j-tile one
            # 2-bank [128,1024] scores tile spanning both i-chunks: one exp
            # per j-tile over the whole valid column range, and the K/V
            # weight loads shared by the paired matmuls. o_ps0 (i<512)
            # finishes at jt=3; its normalize overlaps the jt>=4 tail. ----
            def head(h):
                nt, po = h // 2, (h % 2) * 64
                o_ps0 = op.tile([128, 512], f32, tag="oaug")
                o_ps1 = op.tile([128, 512], f32, tag="oaug")
                for jt in range(NJ):
                    d0 = jt * 128  # first valid (global) column of this j-tile
                    s2 = sp.tile([128, 1024], f32, tag="scores")
                    p2 = ppool.tile([128, 1024], bf16, tag="probs")
                    lhsK = KS[po : po + 64, nt * T + jt * 128 : nt * T + jt * 128 + 128]
                    if jt < 4:  # contributes to both i-chunks
                        nc.tensor.matmul(
                            s2[:, d0:512],
                            lhsT=lhsK,
                            rhs=QS[po : po + 64, nt * T + d0 : nt * T + 512],
                            start=True,
                            stop=True,
                        )
                        nc.tensor.matmul(
                            s2[:, 512:1024],
                            lhsT=lhsK,
                            rhs=QS[po : po + 64, nt * T + 512 : nt * T + 1024],
                            start=True,
                            stop=True,
                        )
                    else:
                        nc.tensor.matmul(
                            s2[:, d0:1024],
                            lhsT=lhsK,
                            rhs=QS[po : po + 64, nt * T + d0 : nt * T + 1024],
                            start=True,
                            stop=True,
                        )
                    nc.scalar.activation(p2[:, d0:1024], s2[:, d0:1024], Exp, scale=SCALE)
                    nc.gpsimd.affine_select(  # causal triangle on diag block
                        p2[:, d0 : d0 + 128],
                        p2[:, d0 : d0 + 128],
                        pattern=[[1, 128]],
                        compare_op=mybir.AluOpType.is_ge,
                        fill=0.0,
                        base=0,
                        channel_multiplier=-1,
                    )
                    lhsV = VS[:, jt * VW + h * 128 : jt * VW + h * 128 + 128]
                    if jt < 4:
                        nc.tensor.matmul(
                            o_ps0[:, d0:512],
                            lhsT=lhsV,
                            rhs=p2[:, d0:512],
                            start=(jt == 0),
                            stop=(jt == 3),
                            skip_group_check=True,
                        )
                    nc.tensor.matmul(
                        o_ps1[:, max(0, d0 - 512) : 512],
                        lhsT=lhsV,
                        rhs=p2[:, max(512, d0) : 1024],
                        start=(jt == 0),
                        stop=(jt == NJ - 1),
                        skip_group_check=True,
                    )
                    if jt == 3:
                        normalize(o_ps0, h, 0)
                normalize(o_ps1, h, 1)

            # Interleave: each nt's Q/K projection feeds its two heads; the
            # next nt's projection matmuls keep PE busy while ScalarE runs
            # this pair's exps.
            for nt in range(NT):
                qk_proj(nt)
                head(2 * nt)
                head(2 * nt + 1)

            # ---- output projection: lhsT = O'[et, iblk] -> out[i, n] directly
            for it in range(NJ):
                fin = fpool.tile([128, E], f32, tag="fin")
                for n0, nw in ((0, 512), (512, 256)):
                    f_ps = pp.tile([128, 512], f32, tag="proj")
                    for et in range(NT):
                        nc.tensor.matmul(
                            f_ps[:, :nw],
                            lhsT=OS[:, et * T + it * 128 : et * T + it * 128 + 128],
                            rhs=WO[:, et * E + n0 : et * E + n0 + nw],
                            start=(et == 0),
                            stop=False,
                        )
                    nc.tensor.matmul(  # += 1 x bo
                        f_ps[:, :nw],
                        lhsT=ONE[:, 0:128],
                        rhs=BOR[:, n0 : n0 + nw],
                        start=False,
                        stop=True,
                    )
                    nc.scalar.copy(fin[:, n0 : n0 + nw], f_ps[:, :nw])
                    nc.sync.dma_start(
                        out[it * 128 : (it + 1) * 128, n0 : n0 + nw],
                        fin[:, n0 : n0 + nw],
                    )

    nc.compile()
    return nc


def _get_nc():
    if "nc" not in _CACHE:
        _CACHE["nc"] = _build()
    return _CACHE["nc"]


def _make_in_maps(inputs):
    bf = ml_dtypes.bfloat16
    x = np.asarray(inputs["x"], np.float32)
    shared = {
        "wq": np.asarray(inputs["Wq"], np.float32).astype(bf),
        "wk": np.asarray(inputs["Wk"], np.float32).astype(bf),
        "wv": np.asarray(inputs["Wv"], np.float32).astype(bf),
        "wo": np.asarray(inputs["Wo"], np.float32).astype(bf),
        "bqt": np.ascontiguousarray(
            np.asarray(inputs["bq"], np.float32).reshape(NT, 128).T
        ),
        "bkt": np.ascontiguousarray(
            np.asarray(inputs["bk"], np.float32).reshape(NT, 128).T
        ),
        "bvr": np.asarray(inputs["bv"], np.float32).reshape(1, E).astype(bf),
        "bor": np.asarray(inputs["bo"], np.float32).reshape(1, E).astype(bf),
    }
    return [dict(shared, xt=x[b].T.astype(bf)) for b in range(B)]


def _run(inputs, trace=False):
    from concourse import bass_utils

    nc = _get_nc()
    res = bass_utils.run_bass_kernel_spmd(
        nc, _make_in_maps(inputs), core_ids=list(range(B)), trace=trace
    )
    out = np.stack([np.asarray(res.results[c]["out"]) for c in range(B)])
    return out, res


def kernel(**inputs) -> np.ndarray:
    out, _ = _run(inputs, trace=False)
    return out


# revision 13
# speedup vs baseline: 1.0694x; 1.0694x over previous
"""CLIP causal attention (B=8, T=1024, E=768, H=12) on 8 TRN2 NeuronCores.

Strategy: pure data-parallel over batch — core b handles x[b] end to end,
no collectives. All compute in transposed space (embed on partitions):

  X' = x_b^T                       [768, 1024]  (host pre-transposed, bf16)
  Q' = Wq^T @ X' (+bq)             [768, 1024]  lhsT = Wq as stored
  K' = Wk^T @ X' (+bk)             [768, 1024]
  V  = X'^T @ Wv (+bv)             [1024, 768]  lhsT = X' blocks (j on partitions)
  per head h (KQ orientation, j on partitions, i free):
     S'[j,i] = K'_h[:,jblk]^T @ Q'_h          (K=64)
     P' = exp(S' * 1/8)  (no max-subtraction: |S'/8| <= ~7, exact-safe)
     causal: skip fully-masked blocks, restrict to valid cols, tri-mask diag
     O_aug[d,i] = sum_j Vaug_h[j,d]^T @ P'    (Vaug has a ones column ->
                                               row 64 = softmax denominator)
     O'_h = O_aug[0:64] * broadcast(1/denom)
  out = (O'^T @ Wo) + bo           [1024, 768]  lhsT = O' blocks -> direct
                                                untransposed output

Q/K projections are interleaved with the attention heads that consume them:
projection matmuls (PE-only) fill the TensorE while ScalarE streams the exp()
of earlier heads, keeping PE duty high enough that the HAM clock gate stays
at 2.4GHz. All matmul operands bf16 (fp32 PSUM accumulation); measured
end-to-end rel l2 err vs fp32 reference ~5e-3.
"""

import numpy as np
import ml_dtypes

E = 768
T = 1024
B = 8
H = 12
DH = 64
NT = E // 128          # 6 partition-tiles of the embed dim
NJ = T // 128          # 8 partition-tiles of the token dim
SCALE = DH ** -0.5     # folded into the exp() activation's scale operand
VW = H * 128           # V_aug row width: 12 heads x 128 cols (64 data +
                       # ones col + zero pad so the PV stationary operand
                       # is a full 128x128 block -> fast weight load)

_CACHE = {}


def _build():
    import concourse.bass as bass
    import concourse.tile as tile
    from concourse import bacc, mybir

    f32 = mybir.dt.float32
    bf16 = mybir.dt.bfloat16
    Exp = mybir.ActivationFunctionType.Exp

    nc = bacc.Bacc(
        "TRN2",
        target_bir_lowering=False,
        debug=False,
        enable_asserts=False,
        num_devices=B,
    )

    xt = nc.dram_tensor("xt", [E, T], bf16, kind="ExternalInput").ap()
    wq = nc.dram_tensor("wq", [E, E], bf16, kind="ExternalInput").ap()
    wk = nc.dram_tensor("wk", [E, E], bf16, kind="ExternalInput").ap()
    wv = nc.dram_tensor("wv", [E, E], bf16, kind="ExternalInput").ap()
    wo = nc.dram_tensor("wo", [E, E], bf16, kind="ExternalInput").ap()
    bqt = nc.dram_tensor("bqt", [128, NT], f32, kind="ExternalInput").ap()
    bkt = nc.dram_tensor("bkt", [128, NT], f32, kind="ExternalInput").ap()
    bvr = nc.dram_tensor("bvr", [1, E], bf16, kind="ExternalInput").ap()
    bor = nc.dram_tensor("bor", [1, E], bf16, kind="ExternalInput").ap()
    tri = nc.dram_tensor("tri", [128, 128], bf16, kind="ExternalInput").ap()
    out = nc.dram_tensor("out", [T, E], f32, kind="ExternalOutput").ap()

    with tile.TileContext(nc) as tc:
        with (
            tc.tile_pool(name="const", bufs=1) as cpool,
            tc.tile_pool(name="psb", bufs=6) as ppool,
            tc.tile_pool(name="rsb", bufs=3) as rpool,
            tc.tile_pool(name="rbsb", bufs=3) as rbpool,
            tc.tile_pool(name="fin", bufs=2) as fpool,
            tc.tile_pool(name="pp", bufs=2, space="PSUM") as pp,
            tc.tile_pool(name="sp", bufs=2, space="PSUM") as sp,
            tc.tile_pool(name="op", bufs=2, space="PSUM") as op,
        ):
            XT = cpool.tile([128, NT * T], bf16)     # (kt, i)
            WQ = cpool.tile([128, NT * E], bf16)     # (kt, n)
            WK = cpool.tile([128, NT * E], bf16)
            WV = cpool.tile([128, NT * E], bf16)
            WO = cpool.tile([128, NT * E], bf16)
            QS = cpool.tile([128, NT * T], bf16)     # Q' (nt, i)
            KS = cpool.tile([128, NT * T], bf16)
            VS = cpool.tile([128, NJ * VW], bf16)    # (jt, h*65+d), col 64 = ones
            OS = cpool.tile([128, NT * T], bf16)     # O' (et, i)
            BQ = cpool.tile([128, NT], f32)
            BK = cpool.tile([128, NT], f32)
            BVR = cpool.tile([1, E], bf16)
            BOR = cpool.tile([1, E], bf16)
            TRI = cpool.tile([128, 128], bf16)
            ONE = cpool.tile([1, 128], bf16)

            # ---- input DMAs: tiny constants first (Q/K/V evictions need the
            # biases; don't queue them behind 4.5MB of weights), then
            # per-k-tile splits so compute can start early. WV before WQ/WK:
            # the V projection (which only needs XT + WV) runs first. ----
            nc.sync.dma_start(BQ[:], bqt)
            nc.sync.dma_start(BK[:], bkt)
            nc.sync.dma_start(BVR[:], bvr)
            nc.sync.dma_start(BOR[:], bor)
            nc.sync.dma_start(TRI[:], tri)
            xt3 = xt.rearrange("(k p) i -> p k i", p=128)
            w3 = {
                id(WQ): wq.rearrange("(k p) n -> p k n", p=128),
                id(WK): wk.rearrange("(k p) n -> p k n", p=128),
                id(WV): wv.rearrange("(k p) n -> p k n", p=128),
                id(WO): wo.rearrange("(k p) n -> p k n", p=128),
            }
            for kt in range(NT):
                nc.sync.dma_start(XT[:, kt * T : (kt + 1) * T], xt3[:, kt])
                nc.sync.dma_start(WV[:, kt * E : (kt + 1) * E], w3[id(WV)][:, kt])
            for W in (WQ, WK, WO):
                for kt in range(NT):
                    nc.sync.dma_start(W[:, kt * E : (kt + 1) * E], w3[id(W)][:, kt])
            # ---- PE warmup: dummy matmuls with no DMA dependency so the
            # HAM activity monitor lifts the 1.2GHz cold gate before real
            # work arrives (DUM memset first: it gates the dummies) ----
            DUM = cpool.tile([1, 512], bf16)
            nc.vector.memset(DUM[:], 1.0)
            nc.vector.memset(ONE[:], 1.0)
            w_ps = op.tile([128, 512], f32, tag="oaug")
            for _ in range(24):
                nc.tensor.matmul(
                    w_ps[0:1, :], lhsT=DUM[:, 0:1], rhs=DUM[:], start=True, stop=True
                )
            # V_aug: zero the pad columns once, then set the ones column
            nc.vector.memset(VS[:], 0.0)
            for jt in range(NJ):
                nc.vector.memset(
                    VS[:, jt * VW : (jt + 1) * VW]
                    .rearrange("p (h e) -> p h e", e=128)[:, :, 64:65],
                    1.0,
                )

            # ---- V projection: lhsT = X'[kt, jblk] -> V[j, e] + ones-col layout
            for jt in range(NJ):
                for e0, ew, h0, nh in ((0, 512, 0, 8), (512, 256, 8, 4)):
                    ps = pp.tile([128, 512], f32, tag="proj")
                    for kt in range(NT):
                        nc.tensor.matmul(
                            ps[:, :ew],
                            lhsT=XT[:, kt * T + jt * 128 : kt * T + jt * 128 + 128],
                            rhs=WV[:, kt * E + e0 : kt * E + e0 + ew],
                            start=(kt == 0),
                            stop=False,
                        )
                    nc.tensor.matmul(  # += 1 x bv  (bias along free dim)
                        ps[:, :ew],
                        lhsT=ONE[:, 0:128],
                        rhs=BVR[:, e0 : e0 + ew],
                        start=False,
                        stop=True,
                    )
                    dst = (
                        VS[:, jt * VW + h0 * 128 : jt * VW + (h0 + nh) * 128]
                        .rearrange("p (h e) -> p h e", e=128)[:, :, 0:64]
                    )
                    nc.vector.tensor_copy(
                        dst, ps[:, :ew].rearrange("p (h d) -> p h d", d=64)
                    )

            # ---- Q'/K' projection for one 128-row block nt (2 heads) ----
            def qk_proj(nt):
                for W, Bb, DST in ((WQ, BQ, QS), (WK, BK, KS)):
                    for ic in range(2):
                        ps = pp.tile([128, 512], f32, tag="proj")
                        for kt in range(NT):
                            nc.tensor.matmul(
                                ps[:],
                                lhsT=W[:, kt * E + nt * 128 : kt * E + nt * 128 + 128],
                                rhs=XT[:, kt * T + ic * 512 : kt * T + ic * 512 + 512],
                                start=(kt == 0),
                                stop=(kt == NT - 1),
                            )
                        nc.vector.tensor_scalar_add(
                            DST[:, nt * T + ic * 512 : nt * T + ic * 512 + 512],
                            ps[:],
                            Bb[:, nt : nt + 1],
                        )

            def normalize(o_ps, h, ic):
                # softmax denominators live in row 64 (the V_aug ones column).
                # Full-precision reciprocal costs 3.35us on DVE; the ~18-bit
                # approx is plenty, but its BITWISE_NOT seed needs an SBUF
                # operand on hardware, so stage the PSUM row out first.
                nt, po = h // 2, (h % 2) * 64
                dn = rpool.tile([1, 512], f32, tag="denom")
                nc.vector.tensor_copy(dn[:], o_ps[64:65, :])
                r = rpool.tile([1, 512], f32, tag="recip")
                nc.vector.reciprocal_approx_fast(r[:], dn[:])
                rb = rbpool.tile([64, 512], f32, tag="recipb")
                nc.gpsimd.partition_broadcast(rb[:], r[:])
                nc.vector.tensor_mul(
                    OS[po : po + 64, nt * T + ic * 512 : nt * T + ic * 512 + 512],
                    o_ps[0:64, :],
                    rb[:],
                )

            # ---- attention for one head, KQ orientation. Per j-tile one
            # 2-bank [128,1024] scores tile spanning both i-chunks: one exp
            # per j-tile over the whole valid column range, and the K/V
            # weight loads shared by the paired matmuls. o_ps0 (i<512)
            # finishes at jt=3; its normalize overlaps the jt>=4 tail. ----
            def head(h):
                nt, po = h // 2, (h % 2) * 64
                o_ps0 = op.tile([128, 512], f32, tag="oaug")
                o_ps1 = op.tile([128, 512], f32, tag="oaug")
                for jt in range(NJ):
                    d0 = jt * 128  # first valid (global) column of this j-tile
                    s2 = sp.tile([128, 1024], f32, tag="scores")
                    p2 = ppool.tile([128, 1024], bf16, tag="probs")
                    lhsK = KS[po : po + 64, nt * T + jt * 128 : nt * T + jt * 128 + 128]
                    if jt < 4:  # contributes to both i-chunks
                        nc.tensor.matmul(
                            s2[:, d0:512],
                            lhsT=lhsK,
                            rhs=QS[po : po + 64, nt * T + d0 : nt * T + 512],
                            start=True,
                            stop=True,
                        )
                        nc.tensor.matmul(
                            s2[:, 512:1024],
                            lhsT=lhsK,
                            rhs=QS[po : po + 64, nt * T + 512 : nt * T + 1024],
                            start=True,
                            stop=True,
                        )
                    else:
                        nc.tensor.matmul(
                            s2[:, d0:1024],
                            lhsT=lhsK,
                            rhs=QS[po : po + 64, nt * T + d0 : nt * T + 1024],
                            start=True,
                            stop=True,
                        )
                    nc.scalar.activation(p2[:, d0:1024], s2[:, d0:1024], Exp, scale=SCALE)
                    # causal triangle on the diag block. NOT gpsimd
                    # affine_select: mixing custom-op types on GpSimd forces
                    # MODIFY_POOL_CONFIG switches that stall partition_broadcast
                    nc.vector.tensor_mul(
                        p2[:, d0 : d0 + 128], p2[:, d0 : d0 + 128], TRI[:]
                    )
                    lhsV = VS[:, jt * VW + h * 128 : jt * VW + h * 128 + 128]
                    if jt < 4:
                        nc.tensor.matmul(
                            o_ps0[:, d0:512],
                            lhsT=lhsV,
                            rhs=p2[:, d0:512],
                            start=(jt == 0),
                            stop=(jt == 3),
                            skip_group_check=True,
                        )
                    nc.tensor.matmul(
                        o_ps1[:, max(0, d0 - 512) : 512],
                        lhsT=lhsV,
                        rhs=p2[:, max(512, d0) : 1024],
                        start=(jt == 0),
                        stop=(jt == NJ - 1),
                        skip_group_check=True,
                    )
                    if jt == 3:
                        normalize(o_ps0, h, 0)
                normalize(o_ps1, h, 1)

            # Interleave: each nt's Q/K projection feeds its two heads; the
            # next nt's projection matmuls keep PE busy while ScalarE runs
            # this pair's exps.
            for nt in range(NT):
                qk_proj(nt)
                head(2 * nt)
                head(2 * nt + 1)

            # ---- output projection: lhsT = O'[et, iblk] -> out[i, n] directly
            for it in range(NJ):
                fin = fpool.tile([128, E], f32, tag="fin")
                for n0, nw in ((0, 512), (512, 256)):
                    f_ps = pp.tile([128, 512], f32, tag="proj")
                    for et in range(NT):
                        nc.tensor.matmul(
                            f_ps[:, :nw],
                            lhsT=OS[:, et * T + it * 128 : et * T + it * 128 + 128],
                            rhs=WO[:, et * E + n0 : et * E + n0 + nw],
                            start=(et == 0),
                            stop=False,
                        )
                    nc.tensor.matmul(  # += 1 x bo
                        f_ps[:, :nw],
                        lhsT=ONE[:, 0:128],
                        rhs=BOR[:, n0 : n0 + nw],
                        start=False,
                        stop=True,
                    )
                    nc.scalar.copy(fin[:, n0 : n0 + nw], f_ps[:, :nw])
                    nc.sync.dma_start(
                        out[it * 128 : (it + 1) * 128, n0 : n0 + nw],
                        fin[:, n0 : n0 + nw],
                    )

    nc.compile()
    return nc


def _get_nc():
    if "nc" not in _CACHE:
        _CACHE["nc"] = _build()
    return _CACHE["nc"]


def _make_in_maps(inputs):
    bf = ml_dtypes.bfloat16
    x = np.asarray(inputs["x"], np.float32)
    shared = {
        "wq": np.asarray(inputs["Wq"], np.float32).astype(bf),
        "wk": np.asarray(inputs["Wk"], np.float32).astype(bf),
        "wv": np.asarray(inputs["Wv"], np.float32).astype(bf),
        "wo": np.asarray(inputs["Wo"], np.float32).astype(bf),
        "bqt": np.ascontiguousarray(
            np.asarray(inputs["bq"], np.float32).reshape(NT, 128).T
        ),
        "bkt": np.ascontiguousarray(
            np.asarray(inputs["bk"], np.float32).reshape(NT, 128).T
        ),
        "bvr": np.asarray(inputs["bv"], np.float32).reshape(1, E).astype(bf),
        "bor": np.asarray(inputs["bo"], np.float32).reshape(1, E).astype(bf),
        "tri": np.triu(np.ones((128, 128), np.float32)).astype(bf),
    }
    return [dict(shared, xt=x[b].T.astype(bf)) for b in range(B)]


def _run(inputs, trace=False):
    from concourse import bass_utils

    nc = _get_nc()
    res = bass_utils.run_bass_kernel_spmd(
        nc, _make_in_maps(inputs), core_ids=list(range(B)), trace=trace
    )
    out = np.stack([np.asarray(res.results[c]["out"]) for c in range(B)])
    return out, res


def kernel(**inputs) -> np.ndarray:
    out, _ = _run(inputs, trace=False)
    return out


# revision 15
# speedup vs baseline: 1.1150x; 1.0426x over previous
"""CLIP causal attention (B=8, T=1024, E=768, H=12) on 8 TRN2 NeuronCores.

Strategy: pure data-parallel over batch — core b handles x[b] end to end,
no collectives. All compute in transposed space (embed on partitions):

  X' = x_b^T                       [768, 1024]  (host pre-transposed, bf16)
  Q' = Wq^T @ X' (+bq)             [768, 1024]  lhsT = Wq as stored
  K' = Wk^T @ X' (+bk)             [768, 1024]
  V  = X'^T @ Wv (+bv)             [1024, 768]  lhsT = X' blocks (j on partitions)
  per head h (KQ orientation, j on partitions, i free):
     S'[j,i] = K'_h[:,jblk]^T @ Q'_h          (K=64)
     P' = exp(S' * 1/8)  (no max-subtraction: |S'/8| <= ~7, exact-safe)
     causal: skip fully-masked blocks, restrict to valid cols, tri-mask diag
     O_aug[d,i] = sum_j Vaug_h[j,d]^T @ P'    (Vaug has a ones column ->
                                               row 64 = softmax denominator)
     O'_h = O_aug[0:64] * broadcast(1/denom)
  out = (O'^T @ Wo) + bo           [1024, 768]  lhsT = O' blocks -> direct
                                                untransposed output

Q/K projections are interleaved with the attention heads that consume them:
projection matmuls (PE-only) fill the TensorE while ScalarE streams the exp()
of earlier heads, keeping PE duty high enough that the HAM clock gate stays
at 2.4GHz. All matmul operands bf16 (fp32 PSUM accumulation); measured
end-to-end rel l2 err vs fp32 reference ~5e-3.
"""

import numpy as np
import ml_dtypes

E = 768
T = 1024
B = 8
H = 12
DH = 64
NT = E // 128          # 6 partition-tiles of the embed dim
NJ = T // 128          # 8 partition-tiles of the token dim
SCALE = DH ** -0.5     # folded into the exp() activation's scale operand
VW = H * 128           # V_aug row width: 12 heads x 128 cols (64 data +
                       # ones col + zero pad so the PV stationary operand
                       # is a full 128x128 block -> fast weight load)

_CACHE = {}


def _build():
    import concourse.bass as bass
    import concourse.tile as tile
    from concourse import bacc, mybir

    f32 = mybir.dt.float32
    bf16 = mybir.dt.bfloat16
    Exp = mybir.ActivationFunctionType.Exp

    nc = bacc.Bacc(
        "TRN2",
        target_bir_lowering=False,
        debug=False,
        enable_asserts=False,
        num_devices=B,
    )

    xt = nc.dram_tensor("xt", [E, T], bf16, kind="ExternalInput").ap()
    wq = nc.dram_tensor("wq", [E, E], bf16, kind="ExternalInput").ap()
    wk = nc.dram_tensor("wk", [E, E], bf16, kind="ExternalInput").ap()
    wv = nc.dram_tensor("wv", [E, E], bf16, kind="ExternalInput").ap()
    wo = nc.dram_tensor("wo", [E, E], bf16, kind="ExternalInput").ap()
    bqt = nc.dram_tensor("bqt", [128, NT], f32, kind="ExternalInput").ap()
    bkt = nc.dram_tensor("bkt", [128, NT], f32, kind="ExternalInput").ap()
    bvr = nc.dram_tensor("bvr", [1, E], bf16, kind="ExternalInput").ap()
    bor = nc.dram_tensor("bor", [1, E], bf16, kind="ExternalInput").ap()
    tri = nc.dram_tensor("tri", [128, 128], bf16, kind="ExternalInput").ap()
    out = nc.dram_tensor("out", [T, E], f32, kind="ExternalOutput").ap()

    with tile.TileContext(nc) as tc:
        with (
            tc.tile_pool(name="const", bufs=1) as cpool,
            tc.tile_pool(name="psb", bufs=6) as ppool,
            tc.tile_pool(name="rsb", bufs=3) as rpool,
            tc.tile_pool(name="rbsb", bufs=3) as rbpool,
            tc.tile_pool(name="fin", bufs=2) as fpool,
            tc.tile_pool(name="pp", bufs=2, space="PSUM") as pp,
            tc.tile_pool(name="sp", bufs=2, space="PSUM") as sp,
            tc.tile_pool(name="op", bufs=2, space="PSUM") as op,
        ):
            XT = cpool.tile([128, NT * T], bf16)     # (kt, i)
            WQ = cpool.tile([128, NT * E], bf16)     # (kt, n)
            WK = cpool.tile([128, NT * E], bf16)
            WV = cpool.tile([128, NT * E], bf16)
            WO = cpool.tile([128, NT * E], bf16)
            QS = cpool.tile([128, NT * T], bf16)     # Q' (nt, i)
            KS = cpool.tile([128, NT * T], bf16)
            VS = cpool.tile([128, NJ * VW], bf16)    # (jt, h*65+d), col 64 = ones
            OS = cpool.tile([128, NT * T], bf16)     # O' (et, i)
            BQ = cpool.tile([128, NT], f32)
            BK = cpool.tile([128, NT], f32)
            BVR = cpool.tile([1, E], bf16)
            BOR = cpool.tile([1, E], bf16)
            TRI = cpool.tile([128, 128], bf16)
            ONE = cpool.tile([1, 128], bf16)

            # ---- input DMAs: tiny constants first (Q/K/V evictions need the
            # biases; don't queue them behind 4.5MB of weights), then
            # per-k-tile splits so compute can start early. WV before WQ/WK:
            # the V projection (which only needs XT + WV) runs first. ----
            nc.sync.dma_start(BQ[:], bqt)
            nc.sync.dma_start(BK[:], bkt)
            nc.sync.dma_start(BVR[:], bvr)
            nc.sync.dma_start(BOR[:], bor)
            nc.sync.dma_start(TRI[:], tri)
            BVB = cpool.tile([128, E], bf16)
            FINB = cpool.tile([128, E], bf16)
            nc.gpsimd.partition_broadcast(BVB[:], BVR[:])
            nc.gpsimd.partition_broadcast(FINB[:], BOR[:])
            xt3 = xt.rearrange("(k p) i -> p k i", p=128)
            w3 = {
                id(WQ): wq.rearrange("(k p) n -> p k n", p=128),
                id(WK): wk.rearrange("(k p) n -> p k n", p=128),
                id(WV): wv.rearrange("(k p) n -> p k n", p=128),
                id(WO): wo.rearrange("(k p) n -> p k n", p=128),
            }
            for kt in range(NT):
                nc.sync.dma_start(XT[:, kt * T : (kt + 1) * T], xt3[:, kt])
                nc.sync.dma_start(WV[:, kt * E : (kt + 1) * E], w3[id(WV)][:, kt])
            for W in (WQ, WK, WO):
                for kt in range(NT):
                    nc.sync.dma_start(W[:, kt * E : (kt + 1) * E], w3[id(W)][:, kt])
            # ---- PE warmup: dummy matmuls with no DMA dependency so the
            # HAM activity monitor lifts the 1.2GHz cold gate before real
            # work arrives (DUM memset first: it gates the dummies) ----
            DUM = cpool.tile([1, 512], bf16)
            nc.vector.memset(DUM[:], 1.0)
            nc.vector.memset(ONE[:], 1.0)

            def dummy():
                # dummies allocate from the projection PSUM pool, which is
                # idle during the attention tail; their tile lifetime is one
                # matmul so they never starve the o_ps accumulators
                d_ps = pp.tile([128, 512], f32, tag="proj")
                nc.tensor.matmul(
                    d_ps[0:1, :], lhsT=DUM[:, 0:1], rhs=DUM[:], start=True, stop=True
                )

            for _ in range(24):
                dummy()
            # V_aug: zero the pad columns once, then set the ones column
            nc.vector.memset(VS[:], 0.0)
            for jt in range(NJ):
                nc.vector.memset(
                    VS[:, jt * VW : (jt + 1) * VW]
                    .rearrange("p (h e) -> p h e", e=128)[:, :, 64:65],
                    1.0,
                )

            # ---- V projection: lhsT = X'[kt, jblk] -> V[j, e] + ones-col layout
            for jt in range(NJ):
                for e0, ew, h0, nh in ((0, 512, 0, 8), (512, 256, 8, 4)):
                    ps = pp.tile([128, 512], f32, tag="proj")
                    for kt in range(NT):
                        nc.tensor.matmul(
                            ps[:, :ew],
                            lhsT=XT[:, kt * T + jt * 128 : kt * T + jt * 128 + 128],
                            rhs=WV[:, kt * E + e0 : kt * E + e0 + ew],
                            start=(kt == 0),
                            stop=(kt == NT - 1),
                        )
                    dst = (
                        VS[:, jt * VW + h0 * 128 : jt * VW + (h0 + nh) * 128]
                        .rearrange("p (h e) -> p h e", e=128)[:, :, 0:64]
                    )
                    nc.vector.tensor_add(
                        dst,
                        ps[:, :ew].rearrange("p (h d) -> p h d", d=64),
                        BVB[:, e0 : e0 + ew].rearrange("p (h d) -> p h d", d=64),
                    )

            # ---- Q'/K' projection for one 128-row block nt (2 heads) ----
            def qk_proj(nt):
                for W, Bb, DST in ((WQ, BQ, QS), (WK, BK, KS)):
                    for ic in range(2):
                        ps = pp.tile([128, 512], f32, tag="proj")
                        for kt in range(NT):
                            nc.tensor.matmul(
                                ps[:],
                                lhsT=W[:, kt * E + nt * 128 : kt * E + nt * 128 + 128],
                                rhs=XT[:, kt * T + ic * 512 : kt * T + ic * 512 + 512],
                                start=(kt == 0),
                                stop=(kt == NT - 1),
                            )
                        nc.vector.tensor_scalar_add(
                            DST[:, nt * T + ic * 512 : nt * T + ic * 512 + 512],
                            ps[:],
                            Bb[:, nt : nt + 1],
                        )

            def normalize(o_ps, h, ic):
                # softmax denominators live in row 64 (the V_aug ones column).
                # Full-precision reciprocal costs 3.35us on DVE; the ~18-bit
                # approx is plenty, but its BITWISE_NOT seed needs an SBUF
                # operand on hardware, so stage the PSUM row out first.
                nt, po = h // 2, (h % 2) * 64
                dn = rpool.tile([1, 512], f32, tag="denom")
                nc.vector.tensor_copy(dn[:], o_ps[64:65, :])
                r = rpool.tile([1, 512], f32, tag="recip")
                nc.vector.reciprocal_approx_fast(r[:], dn[:])
                rb = rbpool.tile([64, 512], f32, tag="recipb")
                nc.gpsimd.partition_broadcast(rb[:], r[:])
                nc.vector.tensor_mul(
                    OS[po : po + 64, nt * T + ic * 512 : nt * T + ic * 512 + 512],
                    o_ps[0:64, :],
                    rb[:],
                )

            # ---- attention for one head, KQ orientation. Per j-tile one
            # 2-bank [128,1024] scores tile spanning both i-chunks: one exp
            # per j-tile over the whole valid column range, and the K/V
            # weight loads shared by the paired matmuls. o_ps0 (i<512)
            # finishes at jt=3; its normalize overlaps the jt>=4 tail. ----
            def head(h):
                nt, po = h // 2, (h % 2) * 64
                o_ps0 = op.tile([128, 512], f32, tag="oaug")
                o_ps1 = op.tile([128, 512], f32, tag="oaug")
                for jt in range(NJ):
                    d0 = jt * 128  # first valid (global) column of this j-tile
                    s2 = sp.tile([128, 1024], f32, tag="scores")
                    p2 = ppool.tile([128, 1024], bf16, tag="probs")
                    lhsK = KS[po : po + 64, nt * T + jt * 128 : nt * T + jt * 128 + 128]
                    if jt < 4:  # contributes to both i-chunks
                        nc.tensor.matmul(
                            s2[:, d0:512],
                            lhsT=lhsK,
                            rhs=QS[po : po + 64, nt * T + d0 : nt * T + 512],
                            start=True,
                            stop=True,
                        )
                        nc.tensor.matmul(
                            s2[:, 512:1024],
                            lhsT=lhsK,
                            rhs=QS[po : po + 64, nt * T + 512 : nt * T + 1024],
                            start=True,
                            stop=True,
                        )
                    else:
                        nc.tensor.matmul(
                            s2[:, d0:1024],
                            lhsT=lhsK,
                            rhs=QS[po : po + 64, nt * T + d0 : nt * T + 1024],
                            start=True,
                            stop=True,
                        )
                    if h >= 10:
                        dummy()
                    nc.scalar.activation(p2[:, d0:1024], s2[:, d0:1024], Exp, scale=SCALE)
                    # causal triangle on the diag block. NOT gpsimd
                    # affine_select: mixing custom-op types on GpSimd forces
                    # MODIFY_POOL_CONFIG switches that stall partition_broadcast
                    nc.vector.tensor_mul(
                        p2[:, d0 : d0 + 128], p2[:, d0 : d0 + 128], TRI[:]
                    )
                    lhsV = VS[:, jt * VW + h * 128 : jt * VW + h * 128 + 128]
                    if jt < 4:
                        nc.tensor.matmul(
                            o_ps0[:, d0:512],
                            lhsT=lhsV,
                            rhs=p2[:, d0:512],
                            start=(jt == 0),
                            stop=(jt == 3),
                            skip_group_check=True,
                        )
                    nc.tensor.matmul(
                        o_ps1[:, max(0, d0 - 512) : 512],
                        lhsT=lhsV,
                        rhs=p2[:, max(512, d0) : 1024],
                        start=(jt == 0),
                        stop=(jt == NJ - 1),
                        skip_group_check=True,
                    )
                    if jt == 3:
                        normalize(o_ps0, h, 0)
                normalize(o_ps1, h, 1)

            # Interleave: each nt's Q/K projection feeds its two heads; the
            # next nt's projection matmuls keep PE busy while ScalarE runs
            # this pair's exps.
            for nt in range(NT):
                qk_proj(nt)
                head(2 * nt)
                head(2 * nt + 1)

            # ---- output projection: lhsT = O'[et, iblk] -> out[i, n] directly
            for it in range(NJ):
                fin = fpool.tile([128, E], f32, tag="fin")
                for n0, nw in ((0, 512), (512, 256)):
                    f_ps = pp.tile([128, 512], f32, tag="proj")
                    for et in range(NT):
                        nc.tensor.matmul(
                            f_ps[:, :nw],
                            lhsT=OS[:, et * T + it * 128 : et * T + it * 128 + 128],
                            rhs=WO[:, et * E + n0 : et * E + n0 + nw],
                            start=(et == 0),
                            stop=(et == NT - 1),
                        )
                    nc.vector.tensor_add(
                        fin[:, n0 : n0 + nw], f_ps[:, :nw], FINB[:, n0 : n0 + nw]
                    )
                    nc.sync.dma_start(
                        out[it * 128 : (it + 1) * 128, n0 : n0 + nw],
                        fin[:, n0 : n0 + nw],
                    )

    nc.compile()
    return nc


def _get_nc():
    if "nc" not in _CACHE:
        _CACHE["nc"] = _build()
    return _CACHE["nc"]


def _make_in_maps(inputs):
    bf = ml_dtypes.bfloat16
    x = np.asarray(inputs["x"], np.float32)
    shared = {
        "wq": np.asarray(inputs["Wq"], np.float32).astype(bf),
        "wk": np.asarray(inputs["Wk"], np.float32).astype(bf),
        "wv": np.asarray(inputs["Wv"], np.float32).astype(bf),
        "wo": np.asarray(inputs["Wo"], np.float32).astype(bf),
        "bqt": np.ascontiguousarray(
            np.asarray(inputs["bq"], np.float32).reshape(NT, 128).T
        ),
        "bkt": np.ascontiguousarray(
            np.asarray(inputs["bk"], np.float32).reshape(NT, 128).T
        ),
        "bvr": np.asarray(inputs["bv"], np.float32).reshape(1, E).astype(bf),
        "bor": np.asarray(inputs["bo"], np.float32).reshape(1, E).astype(bf),
        "tri": np.triu(np.ones((128, 128), np.float32)).astype(bf),
    }
    return [dict(shared, xt=x[b].T.astype(bf)) for b in range(B)]


def _run(inputs, trace=False):
    from concourse import bass_utils

    nc = _get_nc()
    res = bass_utils.run_bass_kernel_spmd(
        nc, _make_in_maps(inputs), core_ids=list(range(B)), trace=trace
    )
    out = np.stack([np.asarray(res.results[c]["out"]) for c in range(B)])
    return out, res


def kernel(**inputs) -> np.ndarray:
    out, _ = _run(inputs, trace=False)
    return out


# revision 16
# speedup vs baseline: 1.1239x; 1.0079x over previous
"""CLIP causal attention (B=8, T=1024, E=768, H=12) on 8 TRN2 NeuronCores.

Strategy: pure data-parallel over batch — core b handles x[b] end to end,
no collectives. All compute in transposed space (embed on partitions):

  X' = x_b^T                       [768, 1024]  (host pre-transposed, bf16)
  Q' = Wq^T @ X' (+bq)             [768, 1024]  lhsT = Wq as stored
  K' = Wk^T @ X' (+bk)             [768, 1024]
  V  = X'^T @ Wv (+bv)             [1024, 768]  lhsT = X' blocks (j on partitions)
  per head h (KQ orientation, j on partitions, i free):
     S'[j,i] = K'_h[:,jblk]^T @ Q'_h          (K=64)
     P' = exp(S' * 1/8)  (no max-subtraction: |S'/8| <= ~7, exact-safe)
     causal: skip fully-masked blocks, restrict to valid cols, tri-mask diag
     O_aug[d,i] = sum_j Vaug_h[j,d]^T @ P'    (Vaug has a ones column ->
                                               row 64 = softmax denominator)
     O'_h = O_aug[0:64] * broadcast(1/denom)
  out = (O'^T @ Wo) + bo           [1024, 768]  lhsT = O' blocks -> direct
                                                untransposed output

Q/K projections are interleaved with the attention heads that consume them:
projection matmuls (PE-only) fill the TensorE while ScalarE streams the exp()
of earlier heads, keeping PE duty high enough that the HAM clock gate stays
at 2.4GHz. All matmul operands bf16 (fp32 PSUM accumulation); measured
end-to-end rel l2 err vs fp32 reference ~5e-3.
"""

import numpy as np
import ml_dtypes

E = 768
T = 1024
B = 8
H = 12
DH = 64
NT = E // 128          # 6 partition-tiles of the embed dim
NJ = T // 128          # 8 partition-tiles of the token dim
SCALE = DH ** -0.5     # folded into the exp() activation's scale operand
VW = H * 128           # V_aug row width: 12 heads x 128 cols (64 data +
                       # ones col + zero pad so the PV stationary operand
                       # is a full 128x128 block -> fast weight load)

_CACHE = {}


def _build():
    import concourse.bass as bass
    import concourse.tile as tile
    from concourse import bacc, mybir

    f32 = mybir.dt.float32
    bf16 = mybir.dt.bfloat16
    Exp = mybir.ActivationFunctionType.Exp

    nc = bacc.Bacc(
        "TRN2",
        target_bir_lowering=False,
        debug=False,
        enable_asserts=False,
        num_devices=B,
    )

    xt = nc.dram_tensor("xt", [E, T], bf16, kind="ExternalInput").ap()
    wq = nc.dram_tensor("wq", [E, E], bf16, kind="ExternalInput").ap()
    wk = nc.dram_tensor("wk", [E, E], bf16, kind="ExternalInput").ap()
    wv = nc.dram_tensor("wv", [E, E], bf16, kind="ExternalInput").ap()
    wo = nc.dram_tensor("wo", [E, E], bf16, kind="ExternalInput").ap()
    bqt = nc.dram_tensor("bqt", [128, NT], f32, kind="ExternalInput").ap()
    bkt = nc.dram_tensor("bkt", [128, NT], f32, kind="ExternalInput").ap()
    bvr = nc.dram_tensor("bvr", [1, E], bf16, kind="ExternalInput").ap()
    bor = nc.dram_tensor("bor", [1, E], bf16, kind="ExternalInput").ap()
    tri = nc.dram_tensor("tri", [128, 128], bf16, kind="ExternalInput").ap()
    out = nc.dram_tensor("out", [T, E], f32, kind="ExternalOutput").ap()

    with tile.TileContext(nc) as tc:
        with (
            tc.tile_pool(name="const", bufs=1) as cpool,
            tc.tile_pool(name="psb", bufs=6) as ppool,
            tc.tile_pool(name="rsb", bufs=3) as rpool,
            tc.tile_pool(name="rbsb", bufs=3) as rbpool,
            tc.tile_pool(name="fin", bufs=2) as fpool,
            tc.tile_pool(name="pp", bufs=2, space="PSUM") as pp,
            tc.tile_pool(name="sp", bufs=2, space="PSUM") as sp,
            tc.tile_pool(name="op", bufs=2, space="PSUM") as op,
        ):
            XT = cpool.tile([128, NT * T], bf16)     # (kt, i)
            WQ = cpool.tile([128, NT * E], bf16)     # (kt, n)
            WK = cpool.tile([128, NT * E], bf16)
            WV = cpool.tile([128, NT * E], bf16)
            WO = cpool.tile([128, NT * E], bf16)
            QS = cpool.tile([128, NT * T], bf16)     # Q' (nt, i)
            KS = cpool.tile([128, NT * T], bf16)
            VS = cpool.tile([128, NJ * VW], bf16)    # (jt, h*65+d), col 64 = ones
            OS = cpool.tile([128, NT * T], bf16)     # O' (et, i)
            BQ = cpool.tile([128, NT], f32)
            BK = cpool.tile([128, NT], f32)
            BVR = cpool.tile([1, E], bf16)
            BOR = cpool.tile([1, E], bf16)
            TRI = cpool.tile([128, 128], bf16)
            ONE = cpool.tile([1, 128], bf16)

            # ---- input DMAs: tiny constants first (Q/K/V evictions need the
            # biases; don't queue them behind 4.5MB of weights), then
            # per-k-tile splits so compute can start early. WV before WQ/WK:
            # the V projection (which only needs XT + WV) runs first. ----
            nc.sync.dma_start(BQ[:], bqt)
            nc.sync.dma_start(BK[:], bkt)
            nc.sync.dma_start(BVR[:], bvr)
            nc.sync.dma_start(BOR[:], bor)
            nc.sync.dma_start(TRI[:], tri)
            BVB = cpool.tile([128, E], bf16)
            FINB = cpool.tile([128, E], bf16)
            nc.gpsimd.partition_broadcast(BVB[:], BVR[:])
            nc.gpsimd.partition_broadcast(FINB[:], BOR[:])
            xt3 = xt.rearrange("(k p) i -> p k i", p=128)
            w3 = {
                id(WQ): wq.rearrange("(k p) n -> p k n", p=128),
                id(WK): wk.rearrange("(k p) n -> p k n", p=128),
                id(WV): wv.rearrange("(k p) n -> p k n", p=128),
                id(WO): wo.rearrange("(k p) n -> p k n", p=128),
            }
            for kt in range(NT):
                nc.sync.dma_start(XT[:, kt * T : (kt + 1) * T], xt3[:, kt])
                nc.sync.dma_start(WV[:, kt * E : (kt + 1) * E], w3[id(WV)][:, kt])
            for W in (WQ, WK, WO):
                for kt in range(NT):
                    nc.sync.dma_start(W[:, kt * E : (kt + 1) * E], w3[id(W)][:, kt])
            # ---- PE warmup: dummy matmuls with no DMA dependency so the
            # HAM activity monitor lifts the 1.2GHz cold gate before real
            # work arrives (DUM memset first: it gates the dummies) ----
            DUMW = cpool.tile([128, 128], bf16)
            DUMR = cpool.tile([128, 512], bf16)
            nc.vector.memset(DUMW[:], 1.0)
            nc.vector.memset(DUMR[:], 1.0)
            nc.vector.memset(ONE[:], 1.0)

            def dummy():
                # full-array junk matmul: the HAM activity monitor only lifts
                # the 1.2GHz cold gate for real array occupancy. Allocates
                # from the projection PSUM pool (idle during the attention
                # tail) with a one-matmul lifetime, so it never starves the
                # o_ps accumulators.
                d_ps = pp.tile([128, 512], f32, tag="proj")
                nc.tensor.matmul(
                    d_ps[:], lhsT=DUMW[:], rhs=DUMR[:], start=True, stop=True
                )

            for _ in range(24):
                dummy()
            # V_aug: zero the pad columns once, then set the ones column
            nc.vector.memset(VS[:], 0.0)
            for jt in range(NJ):
                nc.vector.memset(
                    VS[:, jt * VW : (jt + 1) * VW]
                    .rearrange("p (h e) -> p h e", e=128)[:, :, 64:65],
                    1.0,
                )

            # ---- V projection: lhsT = X'[kt, jblk] -> V[j, e] + ones-col layout
            for jt in range(NJ):
                for e0, ew, h0, nh in ((0, 512, 0, 8), (512, 256, 8, 4)):
                    ps = pp.tile([128, 512], f32, tag="proj")
                    for kt in range(NT):
                        nc.tensor.matmul(
                            ps[:, :ew],
                            lhsT=XT[:, kt * T + jt * 128 : kt * T + jt * 128 + 128],
                            rhs=WV[:, kt * E + e0 : kt * E + e0 + ew],
                            start=(kt == 0),
                            stop=(kt == NT - 1),
                        )
                    dst = (
                        VS[:, jt * VW + h0 * 128 : jt * VW + (h0 + nh) * 128]
                        .rearrange("p (h e) -> p h e", e=128)[:, :, 0:64]
                    )
                    nc.vector.tensor_add(
                        dst,
                        ps[:, :ew].rearrange("p (h d) -> p h d", d=64),
                        BVB[:, e0 : e0 + ew].rearrange("p (h d) -> p h d", d=64),
                    )

            # ---- Q'/K' projection for one 128-row block nt (2 heads) ----
            def qk_proj(nt):
                for W, Bb, DST in ((WQ, BQ, QS), (WK, BK, KS)):
                    for ic in range(2):
                        ps = pp.tile([128, 512], f32, tag="proj")
                        for kt in range(NT):
                            nc.tensor.matmul(
                                ps[:],
                                lhsT=W[:, kt * E + nt * 128 : kt * E + nt * 128 + 128],
                                rhs=XT[:, kt * T + ic * 512 : kt * T + ic * 512 + 512],
                                start=(kt == 0),
                                stop=(kt == NT - 1),
                            )
                        nc.vector.tensor_scalar_add(
                            DST[:, nt * T + ic * 512 : nt * T + ic * 512 + 512],
                            ps[:],
                            Bb[:, nt : nt + 1],
                        )

            def normalize(o_ps, h, ic):
                # softmax denominators live in row 64 (the V_aug ones column).
                # Full-precision reciprocal costs 3.35us on DVE; the ~18-bit
                # approx is plenty, but its BITWISE_NOT seed needs an SBUF
                # operand on hardware, so stage the PSUM row out first.
                nt, po = h // 2, (h % 2) * 64
                dn = rpool.tile([1, 512], f32, tag="denom")
                nc.vector.tensor_copy(dn[:], o_ps[64:65, :])
                r = rpool.tile([1, 512], f32, tag="recip")
                nc.vector.reciprocal_approx_fast(r[:], dn[:])
                rb = rbpool.tile([64, 512], f32, tag="recipb")
                nc.gpsimd.partition_broadcast(rb[:], r[:])
                nc.vector.tensor_mul(
                    OS[po : po + 64, nt * T + ic * 512 : nt * T + ic * 512 + 512],
                    o_ps[0:64, :],
                    rb[:],
                )

            # ---- attention for one head, KQ orientation. Per j-tile one
            # 2-bank [128,1024] scores tile spanning both i-chunks: one exp
            # per j-tile over the whole valid column range, and the K/V
            # weight loads shared by the paired matmuls. o_ps0 (i<512)
            # finishes at jt=3; its normalize overlaps the jt>=4 tail. ----
            def head(h):
                nt, po = h // 2, (h % 2) * 64
                o_ps0 = op.tile([128, 512], f32, tag="oaug")
                o_ps1 = op.tile([128, 512], f32, tag="oaug")
                for jt in range(NJ):
                    d0 = jt * 128  # first valid (global) column of this j-tile
                    s2 = sp.tile([128, 1024], f32, tag="scores")
                    p2 = ppool.tile([128, 1024], bf16, tag="probs")
                    lhsK = KS[po : po + 64, nt * T + jt * 128 : nt * T + jt * 128 + 128]
                    if jt < 4:  # contributes to both i-chunks
                        nc.tensor.matmul(
                            s2[:, d0:512],
                            lhsT=lhsK,
                            rhs=QS[po : po + 64, nt * T + d0 : nt * T + 512],
                            start=True,
                            stop=True,
                        )
                        nc.tensor.matmul(
                            s2[:, 512:1024],
                            lhsT=lhsK,
                            rhs=QS[po : po + 64, nt * T + 512 : nt * T + 1024],
                            start=True,
                            stop=True,
                        )
                    else:
                        nc.tensor.matmul(
                            s2[:, d0:1024],
                            lhsT=lhsK,
                            rhs=QS[po : po + 64, nt * T + d0 : nt * T + 1024],
                            start=True,
                            stop=True,
                        )
                    if h >= 8:
                        dummy()
                    nc.scalar.activation(p2[:, d0:1024], s2[:, d0:1024], Exp, scale=SCALE)
                    # causal triangle on the diag block. NOT gpsimd
                    # affine_select: mixing custom-op types on GpSimd forces
                    # MODIFY_POOL_CONFIG switches that stall partition_broadcast
                    nc.vector.tensor_mul(
                        p2[:, d0 : d0 + 128], p2[:, d0 : d0 + 128], TRI[:]
                    )
                    lhsV = VS[:, jt * VW + h * 128 : jt * VW + h * 128 + 128]
                    if jt < 4:
                        nc.tensor.matmul(
                            o_ps0[:, d0:512],
                            lhsT=lhsV,
                            rhs=p2[:, d0:512],
                            start=(jt == 0),
                            stop=(jt == 3),
                            skip_group_check=True,
                        )
                    nc.tensor.matmul(
                        o_ps1[:, max(0, d0 - 512) : 512],
                        lhsT=lhsV,
                        rhs=p2[:, max(512, d0) : 1024],
                        start=(jt == 0),
                        stop=(jt == NJ - 1),
                        skip_group_check=True,
                    )
                    if jt == 3:
                        normalize(o_ps0, h, 0)
                normalize(o_ps1, h, 1)

            # Interleave: each nt's Q/K projection feeds its two heads; the
            # next nt's projection matmuls keep PE busy while ScalarE runs
            # this pair's exps.
            for nt in range(NT):
                qk_proj(nt)
                head(2 * nt)
                head(2 * nt + 1)

            # ---- output projection: lhsT = O'[et, iblk] -> out[i, n] directly
            for it in range(NJ):
                fin = fpool.tile([128, E], f32, tag="fin")
                for n0, nw in ((0, 512), (512, 256)):
                    f_ps = pp.tile([128, 512], f32, tag="proj")
                    for et in range(NT):
                        nc.tensor.matmul(
                            f_ps[:, :nw],
                            lhsT=OS[:, et * T + it * 128 : et * T + it * 128 + 128],
                            rhs=WO[:, et * E + n0 : et * E + n0 + nw],
                            start=(et == 0),
                            stop=(et == NT - 1),
                        )
                    nc.vector.tensor_add(
                        fin[:, n0 : n0 + nw], f_ps[:, :nw], FINB[:, n0 : n0 + nw]
                    )
                    nc.sync.dma_start(
                        out[it * 128 : (it + 1) * 128, n0 : n0 + nw],
                        fin[:, n0 : n0 + nw],
                    )

    nc.compile()
    return nc


def _get_nc():
    if "nc" not in _CACHE:
        _CACHE["nc"] = _build()
    return _CACHE["nc"]


def _make_in_maps(inputs):
    bf = ml_dtypes.bfloat16
    x = np.asarray(inputs["x"], np.float32)
    shared = {
        "wq": np.asarray(inputs["Wq"], np.float32).astype(bf),
        "wk": np.asarray(inputs["Wk"], np.float32).astype(bf),
        "wv": np.asarray(inputs["Wv"], np.float32).astype(bf),
        "wo": np.asarray(inputs["Wo"], np.float32).astype(bf),
        "bqt": np.ascontiguousarray(
            np.asarray(inputs["bq"], np.float32).reshape(NT, 128).T
        ),
        "bkt": np.ascontiguousarray(
            np.asarray(inputs["bk"], np.float32).reshape(NT, 128).T
        ),
        "bvr": np.asarray(inputs["bv"], np.float32).reshape(1, E).astype(bf),
        "bor": np.asarray(inputs["bo"], np.float32).reshape(1, E).astype(bf),
        "tri": np.triu(np.ones((128, 128), np.float32)).astype(bf),
    }
    return [dict(shared, xt=x[b].T.astype(bf)) for b in range(B)]


def _run(inputs, trace=False):
    from concourse import bass_utils

    nc = _get_nc()
    res = bass_utils.run_bass_kernel_spmd(
        nc, _make_in_maps(inputs), core_ids=list(range(B)), trace=trace
    )
    out = np.stack([np.asarray(res.results[c]["out"]) for c in range(B)])
    return out, res


def kernel(**inputs) -> np.ndarray:
    out, _ = _run(inputs, trace=False)
    return out
